# revision 41
# baseline (speedup 1.0000x reference)
"""Trainium2 Bass kernel for nn_ConsciousnessMonitor (histogram_binning).

kernel(**inputs) takes FULL unsharded numpy inputs, returns the full (9,)
float32 output. Shards state_history along the time axis across 8
NeuronCores. Per core the 33.5MB ht shard streams through DMA once
(DMA-bound at the model's 360B/ns), with the masked-mean matmuls using
the ht chunks as the STATIONARY operand and the 8 mask columns as the
MOVING operand - so PE time is ~128 moving columns total and S lands
time-major in half a PSUM bank, needing no later transpose. Min/max are
folded into one PE transpose + max-reduce (min negated), the AllReduce
payload carries invc so the affine (s1,b1) is computed row-replicated
straight off a broadcast DMA read-back. Binning: affine + RNE int cast,
one-hot in bf16 (counts stay exact), joint histograms as bf16 PE
matmuls into one PSUM bank, AllReduce(add), then a batched 4-pair MI
via count-space entropies with compile-time T. The differentiation
branch (variance/cdist, bf16 inputs) is replicated and fully overlapped
with the stream.

Self-contained: shapes/sharding hardcoded; reads no sibling files.
"""
import numpy as np
import ml_dtypes

import concourse.bacc as bacc
import concourse.tile as tile
import concourse.mybir as mybir
from concourse.bass_utils import run_bass_kernel_spmd
from concourse.masks import make_identity

F32 = mybir.dt.float32
I32 = mybir.dt.int32
BF16 = mybir.dt.bfloat16
AX = mybir.AxisListType
OP = mybir.AluOpType
ACT = mybir.ActivationFunctionType

N_CORES = 8
T, D = 32768, 2048
TL = T // N_CORES          # 4096 time steps per core
NB = 10                    # histogram bins per axis
NPAIR = 4                  # partitions (mask pairs)
J = 2 * NPAIR              # 8 masked-mean columns
NDC = D // 128             # 16 contraction chunks
NCH = TL // 128            # 32 time chunks of 128 (PSUM cols / binning)
MEM = 100
SN = 10

LN_T = float(np.log(np.float32(T)))
INV_T = 1.0 / T
EPS_N = T * 1e-10          # joint-count epsilon under common denominator
EPS_RC = float(T) * T * 1e-10  # outer-product epsilon likewise

_CACHE = {}
LAST_RESULTS = None


def _build(debug=False, variant="main"):
    sim1 = variant.startswith("sim1")
    nc = bacc.Bacc("TRN2", target_bir_lowering=False, debug=False,
                   num_devices=1 if sim1 else N_CORES)
    ht = nc.dram_tensor("ht", [D, TL], F32, kind="ExternalInput").ap()
    mmat = nc.dram_tensor("mmat", [128, NDC * J], F32,
                          kind="ExternalInput").ap()
    invc = nc.dram_tensor("invc", [2 * J, 2], F32,
                      kind="ExternalInput").ap()
    memt = nc.dram_tensor("memt", [128, NDC * MEM], BF16,
                          kind="ExternalInput").ap()
    sampt = nc.dram_tensor("sampt", [128, NDC * SN], BF16,
                           kind="ExternalInput").ap()
    out = nc.dram_tensor("out", [9], F32, kind="ExternalOutput").ap()
    if debug:
        dbg_st = nc.dram_tensor("dbg_st", [128, J], F32,
                                kind="ExternalOutput").ap()
        dbg_gmm = nc.dram_tensor("dbg_gmm", [2 * J, 2], F32,
                                 kind="ExternalOutput").ap()
        dbg_s1b1 = nc.dram_tensor("dbg_s1b1", [1, 2 * J], F32,
                                  kind="ExternalOutput").ap()
        dbg_bin = nc.dram_tensor("dbg_bin", [128, 16], I32,
                                 kind="ExternalOutput").ap()
        dbg_gj = nc.dram_tensor("dbg_gj", [NB, NPAIR * NB], F32,
                                kind="ExternalOutput").ap()

    rg = [list(range(N_CORES))]

    with tile.TileContext(nc) as tc:
        with tc.tile_pool(name="consts", bufs=1) as consts, \
             tc.tile_pool(name="sb", bufs=1) as sb, \
             tc.tile_pool(name="htp", bufs=3) as htp, \
             tc.tile_pool(name="psA", bufs=1, space="PSUM") as psA_pool, \
             tc.tile_pool(name="psJ", bufs=1, space="PSUM") as psJ_pool, \
             tc.tile_pool(name="misc", bufs=3, space="PSUM") as misc, \
             tc.tile_pool(name="dram", bufs=1, space="DRAM") as dram:

            # ---- constants / small inputs ----
            ident10 = consts.tile([NB, NB], F32, tag="id10")
            make_identity(nc, ident10[:])
            ident128 = consts.tile([128, 128], F32, tag="id128")
            make_identity(nc, ident128[:])
            ones128 = consts.tile([128, 1], F32, tag="o128")
            nc.gpsimd.memset(ones128[:], 1.0)
            ones10 = consts.tile([NB, 1], F32, tag="o10")
            nc.gpsimd.memset(ones10[:], 1.0)
            ones1_10 = consts.tile([1, NB], F32, tag="o110")
            nc.gpsimd.memset(ones1_10[:], 1.0)
            ones1_128 = consts.tile([1, 128], F32, tag="o1128")
            nc.gpsimd.memset(ones1_128[:], 1.0)
            ones10x10 = consts.tile([NB, NB], F32, tag="o1010")
            nc.gpsimd.memset(ones10x10[:], 1.0)

            cepsrc = consts.tile([NB, 1], F32, tag="cepsrc")
            nc.gpsimd.memset(cepsrc[:], EPS_RC)
            cepsn = consts.tile([NB, 1], F32, tag="cepsn")
            nc.gpsimd.memset(cepsn[:], EPS_N)

            htt0 = htp.tile([128, TL], F32, tag="htt", name="htt")
            nc.sync.dma_start(out=htt0[:], in_=ht[0:128, :])
            m_sb = consts.tile([128, NDC * J], F32, tag="msb")
            nc.sync.dma_start(out=m_sb[:], in_=mmat[:])
            invc_sb = consts.tile([2 * J, 2], F32, tag="invc")
            nc.gpsimd.dma_start(out=invc_sb[:], in_=invc[:])
            mem_sb = consts.tile([128, NDC * MEM], BF16, tag="memsb")
            nc.gpsimd.dma_start(out=mem_sb[:], in_=memt[:])
            samp_sb = consts.tile([128, NDC * SN], BF16, tag="sampsb")
            nc.gpsimd.dma_start(out=samp_sb[:], in_=sampt[:])

            # ---- differentiation branch (all early; overlaps stream) ----
            psG = misc.tile([SN, SN], F32, tag="m")
            for k in range(NDC):
                nc.tensor.matmul(psG[:], samp_sb[:, k * SN:(k + 1) * SN],
                                 samp_sb[:, k * SN:(k + 1) * SN],
                                 start=(k == 0), stop=(k == NDC - 1))
            sqs = sb.tile([128, NDC * SN], F32, tag="sqs")
            nc.vector.tensor_tensor(sqs[:], samp_sb[:], samp_sb[:], OP.mult)
            psr = misc.tile([SN, 1], F32, tag="m")
            for k in range(NDC):
                nc.tensor.matmul(psr[:], sqs[:, k * SN:(k + 1) * SN],
                                 ones128[:], start=(k == 0),
                                 stop=(k == NDC - 1))
            g_sb = sb.tile([SN, SN], F32, tag="gsb")
            nc.scalar.copy(g_sb[:], psG[:])
            r_sb = sb.tile([SN, 1], F32, tag="rsb")
            nc.scalar.copy(r_sb[:], psr[:])

            # variance branch (DVE; early)
            mem3 = mem_sb[:].rearrange("p (k f) -> p k f", f=MEM)
            mean16 = sb.tile([128, NDC], F32, tag="mean16")
            nc.vector.tensor_reduce(mean16[:], mem3, AX.X, OP.add)
            nc.vector.tensor_scalar(mean16[:], mean16[:], 1.0 / MEM, None,
                                    OP.mult)
            cent = sb.tile([128, NDC * MEM], F32, tag="cent")
            nc.vector.tensor_tensor(
                cent[:].rearrange("p (k f) -> p k f", f=MEM), mem3,
                mean16[:, :, None].broadcast_to([128, NDC, MEM]), OP.subtract)
            nc.vector.tensor_tensor(cent[:], cent[:], cent[:], OP.mult)
            var16 = sb.tile([128, NDC], F32, tag="var16")
            nc.vector.tensor_reduce(
                var16[:], cent[:].rearrange("p (k f) -> p k f", f=MEM),
                AX.X, OP.add)
            nc.vector.tensor_scalar(var16[:], var16[:], 1.0 / (MEM - 1), None,
                                    OP.mult)
            redv = sb.tile([128, 1], F32, tag="redv")
            nc.vector.tensor_reduce(redv[:], var16[:], AX.X, OP.add)
            v2 = sb.tile([128, NDC], F32, tag="v2")
            nc.vector.tensor_tensor(v2[:], var16[:], var16[:], OP.mult)
            redv2 = sb.tile([128, 1], F32, tag="redv2")
            nc.vector.tensor_reduce(redv2[:], v2[:], AX.X, OP.add)
            pstv = misc.tile([1, 1], F32, tag="m")
            nc.tensor.matmul(pstv[:], redv[:], ones128[:], start=True,
                             stop=True)
            tv_sb = sb.tile([1, 1], F32, tag="tvsb")
            nc.scalar.copy(tv_sb[:], pstv[:])
            pss2 = misc.tile([1, 1], F32, tag="m")
            nc.tensor.matmul(pss2[:], redv2[:], ones128[:], start=True,
                             stop=True)
            s2_sb = sb.tile([1, 1], F32, tag="s2sb")
            nc.scalar.copy(s2_sb[:], pss2[:])

            tvsq = sb.tile([1, 1], F32, tag="tvsq")
            nc.vector.tensor_tensor(tvsq[:], tv_sb[:], tv_sb[:], OP.mult)
            dden = sb.tile([1, 1], F32, tag="dden")
            nc.vector.scalar_tensor_tensor(dden[:], tvsq[:], 1e-6, s2_sb[:],
                                           OP.mult, OP.add)
            rdden = sb.tile([1, 1], F32, tag="rdden")
            nc.vector.reciprocal(rdden[:], dden[:])
            eff_sb = sb.tile([1, 1], F32, tag="effsb")
            nc.vector.tensor_tensor(eff_sb[:], tvsq[:], rdden[:], OP.mult)

            # cdist tail: d2 = r_i + r_j - 2G
            rrow_ps = misc.tile([1, SN], F32, tag="m")
            nc.tensor.transpose(rrow_ps[:], r_sb[:], ident10[:])
            rrow = sb.tile([1, SN], F32, tag="rrow")
            nc.scalar.copy(rrow[:], rrow_ps[:])
            rB = misc.tile([SN, SN], F32, tag="m")
            nc.tensor.matmul(rB[:], ones1_10[:], rrow[:], start=True,
                             stop=True)
            d2 = sb.tile([SN, SN], F32, tag="d2")
            nc.vector.scalar_tensor_tensor(d2[:], g_sb[:], -2.0, rB[:],
                                           OP.mult, OP.add)
            nc.vector.tensor_scalar(d2[:], d2[:], r_sb[:], 0.0, OP.add,
                                    OP.max)
            dst = sb.tile([SN, SN], F32, tag="dst")
            nc.scalar.activation(dst[:], d2[:], ACT.Sqrt)
            dsum = sb.tile([SN, 1], F32, tag="dsum")
            nc.vector.tensor_reduce(dsum[:], dst[:], AX.X, OP.add)
            psD = misc.tile([1, 1], F32, tag="m")
            nc.tensor.matmul(psD[:], dsum[:], ones10[:], start=True, stop=True)
            avg_sb = sb.tile([1, 1], F32, tag="avgsb")
            nc.vector.tensor_scalar(avg_sb[:], psD[:],
                                    float(1.0 / (SN * (SN - 1) + 1e-6)), None,
                                    OP.mult)
            sqtv = sb.tile([1, 1], F32, tag="sqtv")
            nc.scalar.activation(sqtv[:], tv_sb[:], ACT.Sqrt)
            diff_sb = sb.tile([1, 1], F32, tag="diffsb")
            nc.vector.tensor_tensor(diff_sb[:], sqtv[:], avg_sb[:], OP.mult)
            tanhd = sb.tile([1, 1], F32, tag="tanhd")
            nc.scalar.activation(tanhd[:], diff_sb[:], ACT.Tanh)
            # load the Ln act table right after the last Tanh (input dep on
            # tanhd pins the scheduler) so no table switch hits the tail
            lnwarm = sb.tile([1, 1], F32, tag="lnwarm")
            nc.scalar.activation(lnwarm[:], tanhd[:], ACT.Ln)
            outrow = sb.tile([1, 9], F32, tag="outrow")
            nc.vector.tensor_copy(outrow[:, 1:2], diff_sb[:])
            nc.vector.tensor_copy(outrow[:, 2:3], eff_sb[:])
            nc.vector.tensor_copy(outrow[:, 3:4], tv_sb[:])

            # ---- stage A: stream HT; ht chunks stationary, masks moving ----
            # psAll[:, c*J+j] accumulates S.T[t, j] for t-chunk c: 128 t rows
            # on partitions, all 32 chunks x 8 series in half a PSUM bank.
            psAll = psA_pool.tile([128, NCH * J], F32, tag="sacc")
            for dk in range(NDC):
                if dk == 0:
                    htt = htt0
                elif dk == NDC - 1:
                    # eighths so the tail after the last byte only covers
                    # 4 matmuls
                    htt = htp.tile([128, TL], F32, tag="htt", name="htt")
                    qt = TL // 8
                    for h in range(8):
                        q = nc.sync if h % 2 == 0 else nc.gpsimd
                        q.dma_start(
                            out=htt[:, h * qt:(h + 1) * qt],
                            in_=ht[dk * 128:(dk + 1) * 128,
                                   h * qt:(h + 1) * qt])
                else:
                    htt = htp.tile([128, TL], F32, tag="htt", name="htt")
                    q = nc.sync if (dk % 2 == 0) else nc.gpsimd
                    q.dma_start(out=htt[:],
                                in_=ht[dk * 128:(dk + 1) * 128, :])
                for c in range(NCH):
                    # start zeroes the whole 2KB zero-region (bank), so only
                    # the very first matmul in the bank may carry start=True
                    nc.tensor.matmul(psAll[:, c * J:(c + 1) * J],
                                     htt[:, c * 128:(c + 1) * 128],
                                     m_sb[:, dk * J:(dk + 1) * J],
                                     start=(dk == 0 and c == 0),
                                     stop=(dk == NDC - 1 and c == NCH - 1),
                                     skip_group_check=True)

            # ---- stage B: raw min/max per series, scale, AllReduce(max) ----
            # mxmn cols 0:8 = max, cols 8:16 = -min (so one max-reduce after
            # transpose covers both); AR payload col1 carries invc (constant
            # across cores, so max is the identity on it)
            ps3 = psAll[:].rearrange("p (c j) -> p j c", j=J)
            mxmn = sb.tile([128, 2 * J], F32, tag="mxmn")
            nc.vector.tensor_reduce(mxmn[:, 0:J], ps3, AX.X, OP.max)
            nc.vector.tensor_reduce(mxmn[:, J:2 * J], ps3, AX.X, OP.min,
                                    negate=True)
            psT = misc.tile([2 * J, 128], F32, tag="m", name="psT")
            nc.tensor.transpose(psT[:], mxmn[:], ident128[:])
            minmax = sb.tile([2 * J, 2], F32, tag="minmax")
            nc.vector.tensor_copy(minmax[:, 1:2], invc_sb[:, 1:2])
            tmx = sb.tile([2 * J, 1], F32, tag="tmx")
            nc.vector.tensor_reduce(tmx[:], psT[:], AX.X, OP.max)
            nc.vector.tensor_scalar(minmax[:, 0:1], tmx[:],
                                    invc_sb[:, 0:1], None, OP.mult)
            cbA = dram.tile([2 * J, 2], F32, tag="cba")
            cbB = dram.tile([2 * J, 2], F32, tag="cbb")
            nc.sync.dma_start(out=cbA[:], in_=minmax[:])
            if sim1:
                nc.sync.dma_start(out=cbB[:], in_=cbA[:])
            else:
                nc.gpsimd.collective_compute("AllReduce", OP.max,
                                             replica_groups=rg,
                                             ins=[cbA.opt()],
                                             outs=[cbB.opt()])
            # read back replicated on every partition: grow[p, 2r+c]=cbB[r,c]
            grow = sb.tile([128, 4 * J], F32, tag="grow")
            nc.sync.dma_start(
                out=grow[:],
                in_=cbB[:].rearrange("r c -> (r c)")[None, :]
                .broadcast_to([128, 4 * J]))
            # row-wise: rng = max+(-min); s1 = 10/(rng+1e-6);
            # s1eff = s1*invc; b1 = (-min)*s1 - 0.5
            gmax = grow[:, 0:2 * J:2]
            gnmn = grow[:, 2 * J:4 * J:2]
            ginv = grow[:, 1:2 * J:2]
            rrow = sb.tile([128, J], F32, tag="rrow2")
            nc.vector.scalar_tensor_tensor(rrow[:], gmax, 1e-6, gnmn,
                                           OP.add, OP.add)
            nc.vector.reciprocal(rrow[:], rrow[:])
            s1eff = sb.tile([128, J], F32, tag="s1eff")
            nc.vector.tensor_tensor(s1eff[:], rrow[:], ginv, OP.mult)
            b1row = sb.tile([128, J], F32, tag="b1row")
            nc.vector.tensor_tensor(b1row[:], gnmn, rrow[:], OP.mult)
            nc.vector.tensor_scalar(b1row[:], b1row[:], 10.0, -0.5,
                                    OP.mult, OP.add)

            # ---- stage C: affine + int-cast + clamp + one-hot + joints ----
            binf = sb.tile([128, NCH * J], F32, tag="binf")
            b3 = binf[:].rearrange("p (c j) -> p c j", j=J)
            nc.vector.tensor_tensor(
                b3, psAll[:].rearrange("p (c j) -> p c j", j=J),
                s1eff[:, None, :].broadcast_to([128, NCH, J]), OP.mult)
            binint = sb.tile([128, NCH * J], I32, tag="binint")
            nc.vector.tensor_tensor(
                binint[:].rearrange("p (c j) -> p c j", j=J), b3,
                b1row[:, None, :].broadcast_to([128, NCH, J]),
                OP.add)
            # no explicit clamp: edge bins absorb out-of-range via is_le/is_ge
            ohsb = sb.tile([128, NCH * J * NB], BF16, tag="ohsb")
            oh3 = ohsb[:].rearrange("p (c b) -> p c b", b=NB)
            for b in range(NB):
                eng = nc.vector if b < 7 else nc.gpsimd
                op = (OP.is_le if b == 0 else
                      OP.is_ge if b == NB - 1 else OP.is_equal)
                eng.tensor_scalar(oh3[:, :, b], binint[:], b, None, op)
            # joint histograms: all 4 pairs side by side in one PSUM bank
            psJt = psJ_pool.tile([NB, NPAIR * NB], F32, tag="pj")
            for c in range(NCH):
                for p in range(NPAIR):
                    xa = (c * J + 2 * p) * NB
                    ya = (c * J + 2 * p + 1) * NB
                    nc.tensor.matmul(psJt[:, p * NB:(p + 1) * NB],
                                     ohsb[:, xa:xa + NB],
                                     ohsb[:, ya:ya + NB],
                                     start=(c == 0 and p == 0),
                                     stop=(c == NCH - 1 and p == NPAIR - 1),
                                     skip_group_check=True)
            gjl = sb.tile([NB, NPAIR * NB], F32, tag="gjl")
            nc.vector.tensor_copy(gjl[:], psJt[:])
            cbj = dram.tile([NB, NPAIR * NB], F32, tag="cbj")
            cbj2 = dram.tile([NB, NPAIR * NB], F32, tag="cbj2")
            nc.sync.dma_start(out=cbj[:], in_=gjl[:])
            if sim1:
                nc.sync.dma_start(out=cbj2[:], in_=cbj[:])
            else:
                nc.gpsimd.collective_compute("AllReduce", OP.add,
                                             replica_groups=rg,
                                             ins=[cbj.opt()],
                                             outs=[cbj2.opt()])
            gj = sb.tile([NB, NPAIR * NB], F32, tag="gj")
            nc.sync.dma_start(out=gj[:], in_=cbj2[:])

            # ---- stage D: batched MI over the 4 pairs ----
            # mi_p = (1/T) sum_ij n_ij*(ln(n_ij+EPS_N)+LN_T-ln(r_i*c_j+EPS_RC))
            gj3 = gj[:].rearrange("a (p b) -> a p b", b=NB)
            r4 = sb.tile([NB, NPAIR], F32, tag="r4")
            nc.vector.tensor_reduce(r4[:], gj3, AX.X, OP.add)
            pscB = misc.tile([NB, NPAIR * NB], F32, tag="m", name="pscB")
            nc.tensor.matmul(pscB[:], ones10x10[:], gj[:], start=True,
                             stop=True)
            rc = sb.tile([NB, NPAIR * NB], F32, tag="rc")
            nc.vector.tensor_tensor(
                rc[:].rearrange("a (p b) -> a p b", b=NB),
                pscB[:].rearrange("a (p b) -> a p b", b=NB),
                r4[:, :, None].broadcast_to([NB, NPAIR, NB]), OP.mult)
            lnn = sb.tile([NB, NPAIR * NB], F32, tag="lnn")
            nc.scalar.activation(lnn[:], gj[:], ACT.Ln, bias=cepsn[:])
            lnrc = sb.tile([NB, NPAIR * NB], F32, tag="lnrc")
            nc.scalar.activation(lnrc[:], rc[:], ACT.Ln, bias=cepsrc[:])
            lterm = sb.tile([NB, NPAIR * NB], F32, tag="lterm")
            nc.vector.scalar_tensor_tensor(lterm[:], lnn[:], LN_T, lnrc[:],
                                           OP.add, OP.subtract)
            nc.vector.tensor_tensor(lterm[:], gj[:], lterm[:], OP.mult)
            rsum = sb.tile([NB, NPAIR], F32, tag="rsum")
            nc.vector.tensor_reduce(
                rsum[:], lterm[:].rearrange("a (p b) -> a p b", b=NB),
                AX.X, OP.add)
            psmi = misc.tile([1, NPAIR], F32, tag="m", name="psmi")
            nc.tensor.matmul(psmi[:], ones10[:], rsum[:], start=True,
                             stop=True)
            nc.vector.tensor_scalar(outrow[:, 5:9], psmi[:], INV_T, 0.0,
                                    OP.mult, OP.max)
            nc.vector.tensor_reduce(outrow[:, 4:5], outrow[:, 5:9], AX.X,
                                    OP.min)
            nc.vector.tensor_tensor(outrow[:, 0:1], outrow[:, 4:5], tanhd[:],
                                    OP.add)
            nc.sync.dma_start(out=out[:], in_=outrow[:])
            if debug:
                nc.sync.dma_start(out=dbg_st[:], in_=binf[:, 0:J])
                nc.sync.dma_start(out=dbg_gmm[:], in_=minmax[:])
                nc.sync.dma_start(out=dbg_s1b1[:, 0:J], in_=s1eff[0:1, :])
                nc.sync.dma_start(out=dbg_bin[:], in_=binint[:, 0:16])
                nc.sync.dma_start(out=dbg_gj[:], in_=gj[:])

    nc.compile()
    return nc


def _build_variant(name):
    return _build(variant=name)


def _get_nc(debug=False):
    key = ("ncd" if debug else "nc")
    if key not in _CACHE:
        _CACHE[key] = _build(debug)
    return _CACHE[key]


def kernel(state, state_memory, state_history, partitions, sample_idx,
           trace=False, debug=False):
    global LAST_RESULTS
    state = np.asarray(state, np.float32)
    state_memory = np.asarray(state_memory, np.float32)
    state_history = np.asarray(state_history, np.float32)
    partitions = np.asarray(partitions)
    sample_idx = np.asarray(sample_idx)

    mmat = np.empty((D, J), np.float32)
    invc8 = np.empty((J,), np.float32)
    pf = partitions.astype(np.float32)
    for p in range(NPAIR):
        mmat[:, 2 * p] = pf[p]
        mmat[:, 2 * p + 1] = np.float32(1.0) - pf[p]
        invc8[2 * p] = np.float32(1.0) / pf[p].sum(dtype=np.float32)
        invc8[2 * p + 1] = np.float32(1.0) / (np.float32(1.0) - pf[p]).sum(
            dtype=np.float32)
    invc = np.zeros((2 * J, 2), np.float32)
    invc[:, 0] = np.tile(invc8, 2)
    invc[0:J, 1] = np.float32(10.0) * invc8
    memory = np.concatenate([state, state_memory[state.shape[0]:]], axis=0)

    def _relayout(arrT, f):
        # [D, f] row-major -> [128, NDC*f]: row p holds chunks k at cols k*f
        return np.ascontiguousarray(
            arrT.reshape(NDC, 128, f).transpose(1, 0, 2).reshape(128, NDC * f))

    mmat = _relayout(mmat, J)
    memt = _relayout(np.ascontiguousarray(memory.T), MEM).astype(
        ml_dtypes.bfloat16)
    sampt = _relayout(np.ascontiguousarray(memory[sample_idx].T), SN).astype(
        ml_dtypes.bfloat16)

    in_maps = []
    for c in range(N_CORES):
        htc = np.ascontiguousarray(state_history[c * TL:(c + 1) * TL, :].T)
        in_maps.append({"ht": htc, "mmat": mmat, "invc": invc,
                        "memt": memt, "sampt": sampt})

    nc = _get_nc(debug)
    res = run_bass_kernel_spmd(nc, in_maps, list(range(N_CORES)),
                               trace=trace)
    LAST_RESULTS = res
    return np.asarray(res.results[0]["out"], np.float32)
